# revision 7
# baseline (speedup 1.0000x reference)
"""Trainium2 Bass kernel for nn_CompressSensory (embedding_lookup):
out = twohot_table[argmax(x, axis=1)] for x [1048576, 45] f32.

Strategy: pure data parallel over 8 NeuronCores (131072 rows each). The 45
features decompose by the two-hot table's triangular structure: row idx of
the table = Tri(g-1)+r with set bits at columns 9-g and 9-r. So per input
row it suffices to know the *group* g* and *offset* r* of the argmax:
  - group maxes M_g (9 grouped free-axis reduces over contiguous spans)
  - offset maxes acc_r = max_{g>r} x[Tri(g-1)+r] (8 shifted tensor-tensor
    maxes)
  - row max m, then one-hot compares (M_g == m) -> column 9-g and
    (acc_r == m) -> column 9-r written straight into the output tile.
Exact fp32 equality keeps argmax semantics except for exact ties (the fixed
dataset has one tied row); ties always yield a row-sum != 2, which a host
fixup recomputes exactly.
"""

import os

import numpy as np

# Whole-tile dep granularity: keeps per-instruction sync-wait counts under the
# HW limit (walrus "Too many sync wait commands" on the input DMA otherwise).
os.environ.setdefault("BY_DEFAULT_DISABLE_SUBTILE_DEPS", "1")

import concourse.bass as bass
import concourse.bacc as bacc
import concourse.mybir as mybir
from concourse.tile import TileContext
from concourse.bass_utils import run_bass_kernel_spmd

F32 = mybir.dt.float32
N_CORES = 8
ROWS_TOTAL = 1048576
X_DIM = 45
OUT_DIM = 10
ROWS = ROWS_TOTAL // N_CORES  # 131072 per core
P = 128                       # SBUF partitions
R = 64                        # rows per partition per tile
NTILES = ROWS // (P * R)      # 16
TRI = [g * (g - 1) // 2 for g in range(1, 11)]  # group g occupies [TRI[g-1], TRI[g])

_CACHE = {}


def _build_nc():
    # Bacc (not bare Bass): its finalize() runs generate_event_semaphores,
    # which splits multi-wait DMAs into event-semaphore + 1-wait DMA pairs
    # (walrus rejects DMA pseudo-instructions with >1 sync wait).
    nc = bacc.Bacc()
    x_d = nc.declare_dram_parameter("x", [ROWS, X_DIM], F32, isOutput=False)
    o_d = nc.declare_dram_parameter("o", [ROWS, OUT_DIM], F32, isOutput=True)

    x_v = x_d.rearrange("(n p r) d -> n p (r d)", p=P, r=R)
    o_v = o_d.rearrange("(n p r) e -> n p (r e)", p=P, r=R)

    with TileContext(nc) as tc:
        with tc.tile_pool(name="pool", bufs=3) as pool:
            for n in range(NTILES):
                # bufs=4 so slot reuse (n-4) lands on the same HWDGE lane
                # ((2 DMAs/tile) % 8 lanes): same-lane WAW is FIFO-implicit,
                # keeping this DMA at one sync wait (DIRECT2D's HW limit).
                xt = pool.tile([P, R * X_DIM], F32, tag="xt", bufs=4)
                nc.sync.dma_start(xt[:], x_v[n])
                x3 = xt.rearrange("p (r d) -> p r d", d=X_DIM)

                Mst = pool.tile([P, R * 9], F32, tag="Mst")
                M3 = Mst.rearrange("p (r g) -> p r g", g=9)
                for g in range(1, 10):
                    nc.vector.tensor_reduce(
                        M3[:, :, g - 1], x3[:, :, TRI[g - 1]:TRI[g]],
                        axis=mybir.AxisListType.X, op=mybir.AluOpType.max,
                    )

                acc = pool.tile([P, R * 9], F32, tag="acc")
                a3 = acc.rearrange("p (r g) -> p r g", g=9)
                nc.vector.tensor_copy(a3[:, :, :], x3[:, :, TRI[8]:TRI[9]])
                for g in range(8, 0, -1):
                    nc.vector.tensor_tensor(
                        a3[:, :, 0:g], a3[:, :, 0:g],
                        x3[:, :, TRI[g - 1]:TRI[g]], mybir.AluOpType.max,
                    )

                mrow = pool.tile([P, R], F32, tag="mrow")
                nc.vector.tensor_reduce(
                    mrow[:], a3, axis=mybir.AxisListType.X, op=mybir.AluOpType.max
                )
                m_b9 = mrow.unsqueeze(2).broadcast_to([P, R, 9])
                m_b8 = mrow.unsqueeze(2).broadcast_to([P, R, 8])
                m_b1 = mrow.unsqueeze(2).broadcast_to([P, R, 1])

                ot = pool.tile([P, R * OUT_DIM], F32, tag="ot")
                o3 = ot.rearrange("p (r e) -> p r e", e=OUT_DIM)

                # A-part: col 9-g <- (M_g == m), i.e. cols 8..0 reversed
                nc.vector.tensor_tensor(
                    o3[:, :, 0:9][:, :, ::-1], M3, m_b9, mybir.AluOpType.is_equal
                )
                # B-part r=0 -> col 9
                nc.vector.tensor_tensor(
                    o3[:, :, 9:10], a3[:, :, 0:1], m_b1, mybir.AluOpType.is_equal
                )
                # B-part r=1..8, then merge into cols 8..1
                tmpB = pool.tile([P, R * 8], F32, tag="tmpB")
                t3 = tmpB.rearrange("p (r g) -> p r g", g=8)
                nc.vector.tensor_tensor(
                    t3, a3[:, :, 1:9], m_b8, mybir.AluOpType.is_equal
                )
                nc.vector.tensor_tensor(
                    o3[:, :, 1:9], o3[:, :, 1:9], t3[:, :, ::-1],
                    mybir.AluOpType.add,
                )

                nc.sync.dma_start(o_v[n], ot[:])
    return nc


def _get_nc():
    if "nc" not in _CACHE:
        nc = _build_nc()
        if not nc.is_finalized():
            nc.finalize()  # Bacc: alloc_regs + generate_event_semaphores
        _CACHE["nc"] = nc
    return _CACHE["nc"]


def run_on_hw(x, trace=False, **kw):
    """Run the SPMD kernel on the 8 cores; returns (out [ROWS_TOTAL,10], results)."""
    nc = _get_nc()
    shards = x.reshape(N_CORES, ROWS, X_DIM)
    in_maps = [{"x": np.ascontiguousarray(shards[c])} for c in range(N_CORES)]
    res = run_bass_kernel_spmd(nc, in_maps, list(range(N_CORES)), trace=trace, **kw)
    out = np.concatenate([np.asarray(r["o"]) for r in res.results], axis=0)
    return out, res


def kernel(x, twohot_table):
    x = np.asarray(x, dtype=np.float32)
    table = np.asarray(twohot_table, dtype=np.float32)
    assert x.shape == (ROWS_TOTAL, X_DIM), x.shape

    out, _ = run_on_hw(x)

    # Exact-tie fixup: equality-based argmax sets >2 bits on tied rows
    # (row-sum != 2). Recompute those rows exactly on host.
    bad = out.sum(axis=1) != 2.0
    if bad.any():
        out[bad] = table[x[bad].argmax(axis=1)]
    return out
